# revision 23
# baseline (speedup 1.0000x reference)
"""Multi-head attention (B=2, L=4096, D=512, H=8) on 8 trn2 NeuronCores.

Sharding: core c -> batch b = c//4, head pair (2p, 2p+1) with p = c%4.
Each core computes, for its 2 heads: QKV projections, scores, softmax,
the full attention-probability matrix (written to HBM - the dominant
memory cost), and the row-parallel slice of the output projection.
The host sums the 4 partial output projections per batch (all-reduce of
the row-parallel Wo) and adds bo.

Per-core device kernel (all matmuls bf16, fp32 accumulate):
  phase 1: qhT[128,L] = Wq[sl] @ q^T (+bq), khT likewise, vh[L,128]
           (head pair packed on partitions / free dim).
  Per 1024-wide query block j:
    stream B: S^T = khT^T @ qhT (64x64 quadrant matmuls) -> exp on ACT
        -> PV matmuls (software-pipelined one chunk behind the exp)
        accumulate unnormalized context^T for both heads.
    stream A (per 128-row query block): S = qhT^T @ khT -> exp -> DVE
        row-sum Z + scale by 1/Z -> DMA attn rows to HBM.
    out proj: per head, ctxT^T @ Wo[:,sl]^T scaled by 1/Z -> partial out.
"""

import os
import numpy as np
import ml_dtypes

B, L, D, H = 2, 4096, 512, 8
NCORES = 8
P = 128
DH = 64

LAST_EXEC_NS = None
LAST_PROFILE_DIR = None


def _split_drain_tc(tile, nc):
    """TileContext adapted to this walrus build's limit of 1 sync-wait per
    instruction (4 observed OK only on some CTRL forms, so we use 1
    everywhere): overflow waits move onto single-wait NoOps inserted just
    before the instruction, and the final drain is split into single-wait
    drains."""

    class SplitDrainTileContext(tile.TileContext):
        WAIT_LIMITS = {}
        WAIT_LIMIT_DEFAULT = 1

        def _split_overflow_waits(self):
            from concourse import mybir

            nid = 0
            for f in self.nc.m.functions:
                for bb in f.blocks:
                    insts = bb.instructions
                    out = []
                    for ins in insts:
                        si = getattr(ins, "sync_info", None)
                        waits = list(si.on_wait) if si and si.on_wait else []
                        lim = self.WAIT_LIMITS.get(
                            type(ins).__name__, self.WAIT_LIMIT_DEFAULT)
                        if len(waits) > lim:
                            keep = waits[-lim:] if lim else []
                            extra = waits[:-lim] if lim else waits
                            for c in range(len(extra)):
                                nop = mybir.InstNoOp(
                                    name=f"I-wsplit{nid}", ins=[], outs=[])
                                nid += 1
                                nop.engine = ins.engine
                                nop.sync_info = mybir.SyncInfo(
                                    on_wait=[extra[c]], on_update=[])
                                self.nc.register_instruction(
                                    nop, overwrite=True)
                                out.append(nop)
                            ins.sync_info = mybir.SyncInfo(
                                on_wait=keep, on_update=si.on_update or [])
                        out.append(ins)
                    insts[:] = out

        def _drain_and_barrier(self, tick_clock, wait_clock):
            from concourse.vector_clock import ScopedClock

            vc = tick_clock.global_clock
            entries = []
            for proc in range(len(vc)):
                t = vc[proc]
                if t > 0:
                    entries.append((None, proc, t))
            for s in range(len(entries)):
                d = self.nc.sync.drain()
                req = ScopedClock()
                scope, proc, t = entries[s]
                req.require_at_least(scope, proc, t)
                wait_clock.add_sem_waits(d.ins, req)

            self.nc.all_engine_barrier()
            assert self.sems is not None
            popped = self.nc._tile_sem_poison_stack.pop()
            assert popped is self._sem_poison
            self.nc.clear_and_free_semaphores(list(self.sems.allocated().values()))
            self.nc.all_engine_barrier()
            self._split_overflow_waits()

    return SplitDrainTileContext(nc)


def build_nc(L=L, exp_bufs=3, attn_bufs=3, pt_bufs=4, sc_bufs=3):
    import concourse.bass as bass
    import concourse.tile as tile
    from concourse import mybir

    # Track HWDGE DMA completion with a single semaphore lane (HWDGE is
    # FIFO per issuing engine, so this is sound) - fewer waits per instr.
    import concourse.tile_sem_assignment as tsa
    tsa.NUM_HWDGE_SEMS = 1

    f32 = mybir.dt.float32
    bf16 = mybir.dt.bfloat16
    Exp = mybir.ActivationFunctionType.Exp
    AX = mybir.AxisListType.X

    NQB = L // P        # 128-row query blocks
    NJB = L // 512
    JW = min(1024, L)   # query-block width for the j loop
    NJ4 = L // JW
    NKC = L // P        # 128-row key chunks
    SA_W = min(1024, L)
    NSA = L // SA_W
    NMM = SA_W // 512

    nc = bass.Bass()
    xqT = nc.declare_dram_parameter("xqT", [D, L], bf16, isOutput=False)
    xkT = nc.declare_dram_parameter("xkT", [D, L], bf16, isOutput=False)
    xvT = nc.declare_dram_parameter("xvT", [D, L], bf16, isOutput=False)
    wqT = nc.declare_dram_parameter("wqT", [D, P], bf16, isOutput=False)
    wkT = nc.declare_dram_parameter("wkT", [D, P], bf16, isOutput=False)
    wvT = nc.declare_dram_parameter("wvT", [D, P], bf16, isOutput=False)
    bq = nc.declare_dram_parameter("bq_", [1, P], bf16, isOutput=False)
    bk = nc.declare_dram_parameter("bk_", [1, P], bf16, isOutput=False)
    bv = nc.declare_dram_parameter("bv_", [1, P], bf16, isOutput=False)
    woT = nc.declare_dram_parameter("woT", [P, D], bf16, isOutput=False)
    attn = nc.declare_dram_parameter("attn", [2, L, L], f32, isOutput=True)
    pout = nc.declare_dram_parameter("pout", [L, D], f32, isOutput=True)

    NCH = D // P  # 4 contraction chunks for the projections

    with _split_drain_tc(tile, nc) as tc:
        with tc.tile_pool(name="consts", bufs=1) as consts, \
             tc.tile_pool(name="persist", bufs=1) as persist:
            wq_sb = consts.tile([P, NCH, P], bf16)
            wk_sb = consts.tile([P, NCH, P], bf16)
            wv_sb = consts.tile([P, NCH, P], bf16)
            wo_sb = consts.tile([P, D], bf16)
            bq_sb = consts.tile([1, P], bf16)
            bk_sb = consts.tile([1, P], bf16)
            bv_sb = consts.tile([1, P], bf16)
            ones = consts.tile([1, 512], bf16)
            nc.vector.memset(ones, 1.0)
            for wsb, wdr in ((wq_sb, wqT), (wk_sb, wkT), (wv_sb, wvT)):
                nc.sync.dma_start(
                    wsb[:], wdr.rearrange("(n p) m -> p n m", p=P))
            nc.sync.dma_start(wo_sb[:], woT[:])
            nc.sync.dma_start(bq_sb[:], bq[:])
            nc.sync.dma_start(bk_sb[:], bk[:])
            nc.sync.dma_start(bv_sb[:], bv[:])

            qhT = persist.tile([P, L], bf16)
            khT = persist.tile([P, L], bf16)
            vh = persist.tile([P, NKC, P], bf16)
            ctxT = persist.tile([P, L], bf16)
            rz_all = persist.tile([P, 2, NQB], f32)

            # ---------------- phase 1: projections ----------------
            with tc.tile_pool(name="xload", bufs=5) as xpool, \
                 tc.tile_pool(name="pj", bufs=2, space="PSUM") as pj, \
                 tc.tile_pool(name="pv", bufs=2, space="PSUM") as pvps:
                for wsb, bsb, xdr, dst in (
                    (wq_sb, bq_sb, xqT, qhT),
                    (wk_sb, bk_sb, xkT, khT),
                ):
                    xs = []
                    for c in range(NCH):
                        xt = xpool.tile([P, L], bf16, tag="x", name=f"x{c}")
                        nc.sync.dma_start(xt[:], xdr[c * P:(c + 1) * P, :])
                        xs.append(xt)
                    for jb in range(NJB):
                        ps = pj.tile([P, 512], f32)
                        for c in range(NCH):
                            nc.tensor.matmul(
                                ps[:],
                                lhsT=wsb[:, c, :],
                                rhs=xs[c][:, jb * 512:(jb + 1) * 512],
                                start=(c == 0), stop=False)
                        nc.tensor.matmul(
                            ps[:], lhsT=bsb[:], rhs=ones[:],
                            start=False, stop=True)
                        nc.vector.tensor_copy(
                            dst[:, jb * 512:(jb + 1) * 512], ps[:])
                xs = []
                for c in range(NCH):
                    xt = xpool.tile([P, L], bf16, tag="x", name=f"xv{c}")
                    nc.sync.dma_start(xt[:], xvT[c * P:(c + 1) * P, :])
                    xs.append(xt)
                for kc in range(NKC):
                    ps = pvps.tile([P, P], f32)
                    for c in range(NCH):
                        nc.tensor.matmul(
                            ps[:],
                            lhsT=xs[c][:, kc * P:(kc + 1) * P],
                            rhs=wv_sb[:, c, :],
                            start=(c == 0), stop=False)
                    nc.tensor.matmul(
                        ps[:], lhsT=ones[:, :P], rhs=bv_sb[:],
                        start=False, stop=True)
                    nc.vector.tensor_copy(vh[:, kc, :], ps[:])

            # ---------------- phase 2: attention + out proj ----------------
            with tc.tile_pool(name="sc", bufs=sc_bufs, space="PSUM") as sc_ps, \
                 tc.tile_pool(name="ct", bufs=1, space="PSUM") as ct_ps, \
                 tc.tile_pool(name="exps", bufs=exp_bufs) as exp_pool, \
                 tc.tile_pool(name="attno", bufs=attn_bufs) as attn_pool, \
                 tc.tile_pool(name="pt", bufs=pt_bufs) as pt_pool, \
                 tc.tile_pool(name="ob", bufs=6) as out_pool, \
                 tc.tile_pool(name="zp", bufs=4) as z_pool:
                for j in range(NJ4):
                    # ---- stream B: unnormalized context for cols j*JW ----
                    ct = ct_ps.tile([P, JW], f32, tag="ct", name="ct")

                    def emit_pv(kc, pts):
                        for t in range(JW // 512):
                            for hh in range(2):
                                hp = DH * hh
                                nc.tensor.matmul(
                                    ct[hp:hp + DH, t * 512:(t + 1) * 512],
                                    lhsT=vh[:, kc, hp:hp + DH],
                                    rhs=pts[hh][:, t * 512:(t + 1) * 512],
                                    start=(kc == 0), stop=(kc == NKC - 1),
                                    skip_group_check=True)

                    prev = None
                    for kc in range(NKC):
                        sts = [sc_ps.tile([P, JW], f32, tag="sc",
                                          name=f"st{hh}") for hh in range(2)]
                        for t in range(JW // 512):
                            for hh in range(2):
                                hp = DH * hh
                                for ih in range(2):
                                    nc.tensor.matmul(
                                        sts[hh][DH * ih:DH * ih + DH,
                                                t * 512:(t + 1) * 512],
                                        lhsT=khT[hp:hp + DH, kc * P + DH * ih:kc * P + DH * ih + DH],
                                        rhs=qhT[hp:hp + DH, j * JW + t * 512:j * JW + (t + 1) * 512],
                                        start=True, stop=True)
                        pts = []
                        for hh in range(2):
                            pt = pt_pool.tile([P, JW], bf16, tag="pt",
                                              name=f"pt{hh}")
                            nc.scalar.activation(pt[:], sts[hh][:], Exp)
                            pts.append(pt)
                        # PV delayed one chunk: its exp is already done, so
                        # the in-order PE queue never stalls on ACT.
                        if prev is not None:
                            emit_pv(kc - 1, prev)
                        prev = pts
                    emit_pv(NKC - 1, prev)
                    nc.vector.tensor_copy(ctxT[:, j * JW:(j + 1) * JW], ct[:])

                    # ---- stream A: attn rows for JW//128 query blocks ----
                    for i2 in range(JW // P):
                        i = j * (JW // P) + i2
                        exps = [exp_pool.tile([P, L], f32, tag="expS",
                                              name=f"expS{hh}")
                                for hh in range(2)]
                        for sch in range(NSA):
                            sas = [sc_ps.tile([P, SA_W], f32, tag="sc",
                                              name=f"sa{hh}") for hh in range(2)]
                            for t in range(NMM):
                                kk = t * 512
                                for hh in range(2):
                                    hp = DH * hh
                                    for qh2 in range(2):
                                        nc.tensor.matmul(
                                            sas[hh][DH * qh2:DH * qh2 + DH,
                                                    kk:kk + 512],
                                            lhsT=qhT[hp:hp + DH, i * P + DH * qh2:i * P + DH * qh2 + DH],
                                            rhs=khT[hp:hp + DH, sch * SA_W + kk:sch * SA_W + kk + 512],
                                            start=True, stop=True)
                            for hh in range(2):
                                nc.scalar.activation(
                                    exps[hh][:, sch * SA_W:(sch + 1) * SA_W],
                                    sas[hh][:], Exp)
                        for hh in range(2):
                            z = z_pool.tile([P, 1], f32, tag="z",
                                            name=f"z{hh}")
                            nc.vector.reduce_sum(z[:], exps[hh][:], axis=AX)
                            nc.vector.reciprocal(
                                rz_all[:, hh, i:i + 1], z[:, 0:1])
                            ao = attn_pool.tile([P, L], f32, tag="ao",
                                                name=f"ao{hh}")
                            nc.vector.tensor_scalar_mul(
                                ao[:], exps[hh][:], rz_all[:, hh, i:i + 1])
                            nc.sync.dma_start(
                                attn[hh, i * P:(i + 1) * P, :], ao[:])

                    # ---- out projection for this j's query blocks ----
                    for i2 in range(JW // P):
                        i = j * (JW // P) + i2
                        obs = []
                        for hh in range(2):
                            hp = DH * hh
                            po = sc_ps.tile([P, D], f32, tag="sc",
                                            name=f"po{hh}")
                            for qh2 in range(2):
                                nc.tensor.matmul(
                                    po[DH * qh2:DH * qh2 + DH, :],
                                    lhsT=ctxT[hp:hp + DH, i * P + DH * qh2:i * P + DH * qh2 + DH],
                                    rhs=wo_sb[hp:hp + DH, :],
                                    start=True, stop=True)
                            ob = out_pool.tile([P, D], f32, tag="obh",
                                               name=f"obh{hh}")
                            nc.vector.tensor_scalar_mul(
                                ob[:], po[:], rz_all[:, hh, i:i + 1])
                            obs.append(ob)
                        osum = out_pool.tile([P, D], f32, tag="osum",
                                             name="osum")
                        nc.vector.tensor_add(osum[:], obs[0][:], obs[1][:])
                        nc.sync.dma_start(pout[i * P:(i + 1) * P, :], osum[:])

    return nc


def make_in_maps(q, k, v, Wq, bq, Wk, bk, Wv, bv, Wo):
    bf = ml_dtypes.bfloat16
    scale = np.float32(1.0 / np.sqrt(np.float32(DH)))
    xT = {}
    for b in range(B):
        xT[b] = (
            np.ascontiguousarray(q[b].T).astype(bf),
            np.ascontiguousarray(k[b].T).astype(bf),
            np.ascontiguousarray(v[b].T).astype(bf),
        )
    in_maps = []
    for c in range(NCORES):
        b, p = divmod(c, 4)
        sl = slice(P * p, P * (p + 1))
        in_maps.append({
            "xqT": xT[b][0], "xkT": xT[b][1], "xvT": xT[b][2],
            "wqT": (np.ascontiguousarray(Wq[sl].T) * scale).astype(bf),
            "wkT": np.ascontiguousarray(Wk[sl].T).astype(bf),
            "wvT": np.ascontiguousarray(Wv[sl].T).astype(bf),
            "bq_": (bq[sl] * scale).reshape(1, P).astype(bf),
            "bk_": bk[sl].reshape(1, P).astype(bf),
            "bv_": bv[sl].reshape(1, P).astype(bf),
            "woT": np.ascontiguousarray(Wo[:, sl].T).astype(bf),
        })
    return in_maps


def kernel(q, k, v, Wq, bq, Wk, bk, Wv, bv, Wo, bo):
    global LAST_EXEC_NS, LAST_PROFILE_DIR
    from concourse.bass_utils import run_bass_kernel_spmd

    q, k, v = (np.asarray(t, np.float32) for t in (q, k, v))
    Wq, bq, Wk, bk, Wv, bv, Wo, bo = (
        np.asarray(t, np.float32) for t in (Wq, bq, Wk, bk, Wv, bv, Wo, bo))

    nc = build_nc()
    in_maps = make_in_maps(q, k, v, Wq, bq, Wk, bk, Wv, bv, Wo)

    trace = bool(os.environ.get("BASS_KERNEL_TRACE"))
    tmpdir = os.environ.get("BASS_KERNEL_TRACE_DIR") or None
    res = run_bass_kernel_spmd(
        nc, in_maps, list(range(NCORES)), trace=trace, tmpdir=tmpdir)
    LAST_EXEC_NS = res.exec_time_ns
    LAST_PROFILE_DIR = tmpdir

    attn = np.empty((B, H, L, L), np.float32)
    out = np.broadcast_to(bo, (B, L, D)).copy()
    for c in range(NCORES):
        b, p = divmod(c, 4)
        attn[b, 2 * p] = res.results[c]["attn"][0]
        attn[b, 2 * p + 1] = res.results[c]["attn"][1]
        out[b] += res.results[c]["pout"]
    return out, attn


# revision 24
# speedup vs baseline: 1.1426x; 1.1426x over previous
"""Multi-head attention (B=2, L=4096, D=512, H=8) on 8 trn2 NeuronCores.

Sharding: core c -> batch b = c//4, head pair (2p, 2p+1) with p = c%4.
Each core computes, for its 2 heads: QKV projections, scores, softmax,
the full attention-probability matrix (written to HBM - the dominant
memory cost), and the row-parallel slice of the output projection.
The host sums the 4 partial output projections per batch (all-reduce of
the row-parallel Wo) and adds bo.

Per-core device kernel (all matmuls bf16, fp32 accumulate):
  phase 1: qhT[128,L] = Wq[sl] @ q^T (+bq), khT likewise, vh[L,128]
           (head pair packed on partitions / free dim).
  Per 1024-wide query block j:
    stream B: S^T = khT^T @ qhT (64x64 quadrant matmuls) -> exp on ACT
        -> PV matmuls (software-pipelined one chunk behind the exp)
        accumulate unnormalized context^T for both heads.
    stream A (per 128-row query block): S = qhT^T @ khT -> exp -> DVE
        row-sum Z + scale by 1/Z -> DMA attn rows to HBM.
    out proj: per head, ctxT^T @ Wo[:,sl]^T scaled by 1/Z -> partial out.
"""

import os
import numpy as np
import ml_dtypes

B, L, D, H = 2, 4096, 512, 8
NCORES = 8
P = 128
DH = 64

LAST_EXEC_NS = None
LAST_PROFILE_DIR = None


def _split_drain_tc(tile, nc):
    """TileContext adapted to this walrus build's limit of 1 sync-wait per
    instruction (4 observed OK only on some CTRL forms, so we use 1
    everywhere): overflow waits move onto single-wait NoOps inserted just
    before the instruction, and the final drain is split into single-wait
    drains."""

    class SplitDrainTileContext(tile.TileContext):
        WAIT_LIMITS = {}
        WAIT_LIMIT_DEFAULT = 1

        def _split_overflow_waits(self):
            from concourse import mybir

            nid = 0
            for f in self.nc.m.functions:
                for bb in f.blocks:
                    insts = bb.instructions
                    out = []
                    for ins in insts:
                        si = getattr(ins, "sync_info", None)
                        waits = list(si.on_wait) if si and si.on_wait else []
                        lim = self.WAIT_LIMITS.get(
                            type(ins).__name__, self.WAIT_LIMIT_DEFAULT)
                        if len(waits) > lim:
                            keep = waits[-lim:] if lim else []
                            extra = waits[:-lim] if lim else waits
                            for c in range(len(extra)):
                                nop = mybir.InstNoOp(
                                    name=f"I-wsplit{nid}", ins=[], outs=[])
                                nid += 1
                                nop.engine = ins.engine
                                nop.sync_info = mybir.SyncInfo(
                                    on_wait=[extra[c]], on_update=[])
                                self.nc.register_instruction(
                                    nop, overwrite=True)
                                out.append(nop)
                            ins.sync_info = mybir.SyncInfo(
                                on_wait=keep, on_update=si.on_update or [])
                        out.append(ins)
                    insts[:] = out

        def _drain_and_barrier(self, tick_clock, wait_clock):
            from concourse.vector_clock import ScopedClock

            vc = tick_clock.global_clock
            entries = []
            for proc in range(len(vc)):
                t = vc[proc]
                if t > 0:
                    entries.append((None, proc, t))
            for s in range(len(entries)):
                d = self.nc.sync.drain()
                req = ScopedClock()
                scope, proc, t = entries[s]
                req.require_at_least(scope, proc, t)
                wait_clock.add_sem_waits(d.ins, req)

            self.nc.all_engine_barrier()
            assert self.sems is not None
            popped = self.nc._tile_sem_poison_stack.pop()
            assert popped is self._sem_poison
            self.nc.clear_and_free_semaphores(list(self.sems.allocated().values()))
            self.nc.all_engine_barrier()
            self._split_overflow_waits()

    return SplitDrainTileContext(nc)


def build_nc(L=L, exp_bufs=3, attn_bufs=3, pt_bufs=4, sc_bufs=3):
    import concourse.bass as bass
    import concourse.tile as tile
    from concourse import mybir

    # Track HWDGE DMA completion with a single semaphore lane (HWDGE is
    # FIFO per issuing engine, so this is sound) - fewer waits per instr.
    import concourse.tile_sem_assignment as tsa
    tsa.NUM_HWDGE_SEMS = 1

    f32 = mybir.dt.float32
    bf16 = mybir.dt.bfloat16
    Exp = mybir.ActivationFunctionType.Exp
    AX = mybir.AxisListType.X

    NQB = L // P        # 128-row query blocks
    NJB = L // 512
    JW = min(1024, L)   # query-block width for the j loop
    NJ4 = L // JW
    NKC = L // P        # 128-row key chunks
    SA_W = min(1024, L)
    NSA = L // SA_W
    NMM = SA_W // 512

    nc = bass.Bass()
    xqT = nc.declare_dram_parameter("xqT", [D, L], bf16, isOutput=False)
    xkT = nc.declare_dram_parameter("xkT", [D, L], bf16, isOutput=False)
    xvT = nc.declare_dram_parameter("xvT", [D, L], bf16, isOutput=False)
    wqT = nc.declare_dram_parameter("wqT", [D, P], bf16, isOutput=False)
    wkT = nc.declare_dram_parameter("wkT", [D, P], bf16, isOutput=False)
    wvT = nc.declare_dram_parameter("wvT", [D, P], bf16, isOutput=False)
    bq = nc.declare_dram_parameter("bq_", [1, P], bf16, isOutput=False)
    bk = nc.declare_dram_parameter("bk_", [1, P], bf16, isOutput=False)
    bv = nc.declare_dram_parameter("bv_", [1, P], bf16, isOutput=False)
    woT = nc.declare_dram_parameter("woT", [P, D], bf16, isOutput=False)
    attn = nc.declare_dram_parameter("attn", [2, L, L], f32, isOutput=True)
    pout = nc.declare_dram_parameter("pout", [L, D], f32, isOutput=True)

    NCH = D // P  # 4 contraction chunks for the projections

    with _split_drain_tc(tile, nc) as tc:
        with tc.tile_pool(name="consts", bufs=1) as consts, \
             tc.tile_pool(name="persist", bufs=1) as persist:
            wq_sb = consts.tile([P, NCH, P], bf16)
            wk_sb = consts.tile([P, NCH, P], bf16)
            wv_sb = consts.tile([P, NCH, P], bf16)
            wo_sb = consts.tile([P, D], bf16)
            bq_sb = consts.tile([1, P], bf16)
            bk_sb = consts.tile([1, P], bf16)
            bv_sb = consts.tile([1, P], bf16)
            ones = consts.tile([1, 512], bf16)
            nc.vector.memset(ones, 1.0)
            for wsb, wdr in ((wq_sb, wqT), (wk_sb, wkT), (wv_sb, wvT)):
                nc.sync.dma_start(
                    wsb[:], wdr.rearrange("(n p) m -> p n m", p=P))
            nc.sync.dma_start(wo_sb[:], woT[:])
            nc.sync.dma_start(bq_sb[:], bq[:])
            nc.sync.dma_start(bk_sb[:], bk[:])
            nc.sync.dma_start(bv_sb[:], bv[:])

            qhT = persist.tile([P, L], bf16)
            khT = persist.tile([P, L], bf16)
            vh = persist.tile([P, NKC, P], bf16)
            ctxT = persist.tile([P, L], bf16)
            rz_all = persist.tile([P, 2, NQB], f32)

            # ---------------- phase 1: projections ----------------
            with tc.tile_pool(name="xload", bufs=5) as xpool, \
                 tc.tile_pool(name="pj", bufs=2, space="PSUM") as pj, \
                 tc.tile_pool(name="pv", bufs=2, space="PSUM") as pvps:
                for wsb, bsb, xdr, dst in (
                    (wq_sb, bq_sb, xqT, qhT),
                    (wk_sb, bk_sb, xkT, khT),
                ):
                    xs = []
                    for c in range(NCH):
                        xt = xpool.tile([P, L], bf16, tag="x", name=f"x{c}")
                        nc.sync.dma_start(xt[:], xdr[c * P:(c + 1) * P, :])
                        xs.append(xt)
                    for jb in range(NJB):
                        ps = pj.tile([P, 512], f32)
                        for c in range(NCH):
                            nc.tensor.matmul(
                                ps[:],
                                lhsT=wsb[:, c, :],
                                rhs=xs[c][:, jb * 512:(jb + 1) * 512],
                                start=(c == 0), stop=False)
                        nc.tensor.matmul(
                            ps[:], lhsT=bsb[:], rhs=ones[:],
                            start=False, stop=True)
                        nc.vector.tensor_copy(
                            dst[:, jb * 512:(jb + 1) * 512], ps[:])
                xs = []
                for c in range(NCH):
                    xt = xpool.tile([P, L], bf16, tag="x", name=f"xv{c}")
                    nc.sync.dma_start(xt[:], xvT[c * P:(c + 1) * P, :])
                    xs.append(xt)
                for kc in range(NKC):
                    ps = pvps.tile([P, P], f32)
                    for c in range(NCH):
                        nc.tensor.matmul(
                            ps[:],
                            lhsT=xs[c][:, kc * P:(kc + 1) * P],
                            rhs=wv_sb[:, c, :],
                            start=(c == 0), stop=False)
                    nc.tensor.matmul(
                        ps[:], lhsT=ones[:, :P], rhs=bv_sb[:],
                        start=False, stop=True)
                    nc.vector.tensor_copy(vh[:, kc, :], ps[:])

            # ---------------- phase 2: attention + out proj ----------------
            with tc.tile_pool(name="sc", bufs=sc_bufs, space="PSUM") as sc_ps, \
                 tc.tile_pool(name="ct", bufs=1, space="PSUM") as ct_ps, \
                 tc.tile_pool(name="exps", bufs=exp_bufs) as exp_pool, \
                 tc.tile_pool(name="attno", bufs=attn_bufs) as attn_pool, \
                 tc.tile_pool(name="pt", bufs=pt_bufs) as pt_pool, \
                 tc.tile_pool(name="ob", bufs=6) as out_pool, \
                 tc.tile_pool(name="zp", bufs=4) as z_pool:
                for j in range(NJ4):
                    # ---- stream B: unnormalized context for cols j*JW ----
                    ct = ct_ps.tile([P, JW], f32, tag="ct", name="ct")

                    def emit_pv(kc, pts):
                        for t in range(JW // 512):
                            for hh in range(2):
                                hp = DH * hh
                                nc.tensor.matmul(
                                    ct[hp:hp + DH, t * 512:(t + 1) * 512],
                                    lhsT=vh[:, kc, hp:hp + DH],
                                    rhs=pts[hh][:, t * 512:(t + 1) * 512],
                                    start=(kc == 0), stop=(kc == NKC - 1),
                                    skip_group_check=True)

                    prev = None
                    for kc in range(NKC):
                        sts = [sc_ps.tile([P, JW], f32, tag="sc",
                                          name=f"st{hh}") for hh in range(2)]
                        for t in range(JW // 512):
                            for hh in range(2):
                                hp = DH * hh
                                for ih in range(2):
                                    nc.tensor.matmul(
                                        sts[hh][DH * ih:DH * ih + DH,
                                                t * 512:(t + 1) * 512],
                                        lhsT=khT[hp:hp + DH, kc * P + DH * ih:kc * P + DH * ih + DH],
                                        rhs=qhT[hp:hp + DH, j * JW + t * 512:j * JW + (t + 1) * 512],
                                        start=True, stop=True)
                        pts = []
                        for hh in range(2):
                            pt = pt_pool.tile([P, JW], bf16, tag="pt",
                                              name=f"pt{hh}")
                            nc.scalar.activation(pt[:], sts[hh][:], Exp)
                            pts.append(pt)
                        # PV delayed one chunk: its exp is already done, so
                        # the in-order PE queue never stalls on ACT.
                        if prev is not None:
                            emit_pv(kc - 1, prev)
                        prev = pts
                    emit_pv(NKC - 1, prev)
                    nc.vector.tensor_copy(ctxT[:, j * JW:(j + 1) * JW], ct[:])

                    # ---- stream A: attn rows for JW//128 query blocks ----
                    for i2 in range(JW // P):
                        i = j * (JW // P) + i2
                        exps = [exp_pool.tile([P, L], f32, tag="expS",
                                              name=f"expS{hh}")
                                for hh in range(2)]
                        zaccs = [z_pool.tile([P, max(NSA, 2)], f32,
                                             tag="zacc", name=f"zacc{hh}")
                                 for hh in range(2)]
                        for sch in range(NSA):
                            sas = [sc_ps.tile([P, SA_W], f32, tag="sc",
                                              name=f"sa{hh}") for hh in range(2)]
                            for t in range(NMM):
                                kk = t * 512
                                for hh in range(2):
                                    hp = DH * hh
                                    for qh2 in range(2):
                                        nc.tensor.matmul(
                                            sas[hh][DH * qh2:DH * qh2 + DH,
                                                    kk:kk + 512],
                                            lhsT=qhT[hp:hp + DH, i * P + DH * qh2:i * P + DH * qh2 + DH],
                                            rhs=khT[hp:hp + DH, sch * SA_W + kk:sch * SA_W + kk + 512],
                                            start=True, stop=True)
                            for hh in range(2):
                                nc.scalar.activation(
                                    exps[hh][:, sch * SA_W:(sch + 1) * SA_W],
                                    sas[hh][:], Exp,
                                    accum_out=zaccs[hh][:, sch:sch + 1])
                        for hh in range(2):
                            z = z_pool.tile([P, 1], f32, tag="z",
                                            name=f"z{hh}")
                            nc.vector.reduce_sum(z[:], zaccs[hh][:, :NSA],
                                                 axis=AX)
                            nc.vector.reciprocal(
                                rz_all[:, hh, i:i + 1], z[:, 0:1])
                            ao = attn_pool.tile([P, L], f32, tag="ao",
                                                name=f"ao{hh}")
                            nc.vector.tensor_scalar_mul(
                                ao[:], exps[hh][:], rz_all[:, hh, i:i + 1])
                            nc.sync.dma_start(
                                attn[hh, i * P:(i + 1) * P, :], ao[:])

                    # ---- out projection for this j's query blocks ----
                    for i2 in range(JW // P):
                        i = j * (JW // P) + i2
                        obs = []
                        for hh in range(2):
                            hp = DH * hh
                            po = sc_ps.tile([P, D], f32, tag="sc",
                                            name=f"po{hh}")
                            for qh2 in range(2):
                                nc.tensor.matmul(
                                    po[DH * qh2:DH * qh2 + DH, :],
                                    lhsT=ctxT[hp:hp + DH, i * P + DH * qh2:i * P + DH * qh2 + DH],
                                    rhs=wo_sb[hp:hp + DH, :],
                                    start=True, stop=True)
                            ob = out_pool.tile([P, D], f32, tag="obh",
                                               name=f"obh{hh}")
                            nc.vector.tensor_scalar_mul(
                                ob[:], po[:], rz_all[:, hh, i:i + 1])
                            obs.append(ob)
                        osum = out_pool.tile([P, D], f32, tag="osum",
                                             name="osum")
                        nc.vector.tensor_add(osum[:], obs[0][:], obs[1][:])
                        nc.sync.dma_start(pout[i * P:(i + 1) * P, :], osum[:])

    return nc


def make_in_maps(q, k, v, Wq, bq, Wk, bk, Wv, bv, Wo):
    bf = ml_dtypes.bfloat16
    scale = np.float32(1.0 / np.sqrt(np.float32(DH)))
    xT = {}
    for b in range(B):
        xT[b] = (
            np.ascontiguousarray(q[b].T).astype(bf),
            np.ascontiguousarray(k[b].T).astype(bf),
            np.ascontiguousarray(v[b].T).astype(bf),
        )
    in_maps = []
    for c in range(NCORES):
        b, p = divmod(c, 4)
        sl = slice(P * p, P * (p + 1))
        in_maps.append({
            "xqT": xT[b][0], "xkT": xT[b][1], "xvT": xT[b][2],
            "wqT": (np.ascontiguousarray(Wq[sl].T) * scale).astype(bf),
            "wkT": np.ascontiguousarray(Wk[sl].T).astype(bf),
            "wvT": np.ascontiguousarray(Wv[sl].T).astype(bf),
            "bq_": (bq[sl] * scale).reshape(1, P).astype(bf),
            "bk_": bk[sl].reshape(1, P).astype(bf),
            "bv_": bv[sl].reshape(1, P).astype(bf),
            "woT": np.ascontiguousarray(Wo[:, sl].T).astype(bf),
        })
    return in_maps


def kernel(q, k, v, Wq, bq, Wk, bk, Wv, bv, Wo, bo):
    global LAST_EXEC_NS, LAST_PROFILE_DIR
    from concourse.bass_utils import run_bass_kernel_spmd

    q, k, v = (np.asarray(t, np.float32) for t in (q, k, v))
    Wq, bq, Wk, bk, Wv, bv, Wo, bo = (
        np.asarray(t, np.float32) for t in (Wq, bq, Wk, bk, Wv, bv, Wo, bo))

    nc = build_nc()
    in_maps = make_in_maps(q, k, v, Wq, bq, Wk, bk, Wv, bv, Wo)

    trace = bool(os.environ.get("BASS_KERNEL_TRACE"))
    tmpdir = os.environ.get("BASS_KERNEL_TRACE_DIR") or None
    res = run_bass_kernel_spmd(
        nc, in_maps, list(range(NCORES)), trace=trace, tmpdir=tmpdir)
    LAST_EXEC_NS = res.exec_time_ns
    LAST_PROFILE_DIR = tmpdir

    attn = np.empty((B, H, L, L), np.float32)
    out = np.broadcast_to(bo, (B, L, D)).copy()
    for c in range(NCORES):
        b, p = divmod(c, 4)
        attn[b, 2 * p] = res.results[c]["attn"][0]
        attn[b, 2 * p + 1] = res.results[c]["attn"][1]
        out[b] += res.results[c]["pout"]
    return out, attn
